# revision 10
# baseline (speedup 1.0000x reference)
"""Trainium2 Bass kernel for nn_CrossModalConceptAttention.

Math: with q_len = kv_len = 1 the attention softmax over a single key is
identically 1, so each branch is an affine map of ctx = concat(cxr, ecg, ehr):

    y_e   = x_e + out_w_e @ (Wv_e @ (kv_w_e @ ctx + kv_b_e) + bv_e) + out_b_e
    out_e = LN(y_e) * g_e + b_e

Everything up to the variance normalization is linear, including the LN mean
subtraction (P = I - 11^T/E).  We fold the whole thing host-side into one
42 -> 42 matrix per branch (scaled by 1/sqrt(E) so the segment sum-of-squares
equals the variance directly), stack the three branches, and run on device:

    per row r:  d = D2 @ ctx_r + c2          (one small matmul)
                var_s = sum(d[seg_s]^2)      (segmented reduce)
                out   = d * Kseg / sqrt(var_s + eps)

Device mapping (per core, rows sharded 8 ways, row p*NJ+j on partition p):
  MM1  (PE):  transpose 3 row-groups of 128 rows at once:
              lhsT = X[128, 126] (42 feats x 3 j), rhs = I[128,128]
              -> X^T stacked [126, 128] in PSUM
  copy (ACT): PSUM -> SBUF, below a persistent row of ones (homogeneous coord)
  MM2  (PE):  lhsT = X^T[127, 128], rhs = D3[127, 144] (block-diag w/ bias row)
              -> d row-major [128 rows, 3 j x 48] in PSUM (48 = 16-padded segs)
  sq   (ACT): square
  red  (DVE): segmented reduce [128, 27, 16] -> [128, 27]
  sqrt (ACT): sigma = sqrt(ssq + eps)   (batched over 4 superunits)
  rcp  (DVE): 1/sigma
  K    (DVE): inv2 = (1/sigma) * Kseg
  mul  (DVE): out = d * broadcast(inv2)  -> SBUF, DMA out (host strips pads)
"""

import numpy as np

B = 2097152
NCORES = 8
BC = B // NCORES            # 262144 rows per core
P = 128                     # SBUF partitions
NJ = BC // P                # 2048 row-groups ("j") per partition
CXR, ECG, EHR = 15, 14, 13
F = CXR + ECG + EHR         # 42 input features
W = 45                      # padded output row width (15 per modality segment)
LN_EPS = 1e-5
J_SLAB = 216                # j-groups per DMA slab
B_STATS = 4                 # superunits per stats batch

_CACHE = {}


def _fold_branch(E, in_w, in_b, out_w, out_b, kv_w, kv_b, lo):
    """Return (D2 rows [E,42], c2 [E]) for one branch, fp64."""
    Wv = in_w[2 * E:].astype(np.float64)
    bv = in_b[2 * E:].astype(np.float64)
    ow = out_w.astype(np.float64)
    ob = out_b.astype(np.float64)
    kw = kv_w.astype(np.float64)
    kb = kv_b.astype(np.float64)
    A = ow @ Wv @ kw                       # [E, 42]
    c = ow @ (Wv @ kb + bv) + ob           # [E]
    M = A.copy()
    M[:, lo:lo + E] += np.eye(E)           # residual x_e
    Pc = np.eye(E) - 1.0 / E               # LN centering
    s = 1.0 / np.sqrt(E)                   # so sum(d^2) == var
    return (Pc @ M) * s, (Pc @ c) * s


def _host_consts(inputs):
    segs = [("cxr", CXR, 0), ("ecg", ECG, CXR), ("ehr", EHR, CXR + ECG)]
    D2 = np.zeros((F, F))
    c2 = np.zeros(F)
    gs, bs = [], []
    for name, E, lo in segs:
        D2[lo:lo + E], c2[lo:lo + E] = _fold_branch(
            E,
            np.asarray(inputs[f"in_w_{name}"]), np.asarray(inputs[f"in_b_{name}"]),
            np.asarray(inputs[f"out_w_{name}"]), np.asarray(inputs[f"out_b_{name}"]),
            np.asarray(inputs[f"kv_w_{name}"]), np.asarray(inputs[f"kv_b_{name}"]),
            lo)
        gs.append(np.asarray(inputs[f"ln_g_{name}"], dtype=np.float64))
        bs.append(np.asarray(inputs[f"ln_b_{name}"], dtype=np.float64))

    # fast path requires per-segment-uniform ln_g and zero ln_b (true for
    # this model's setup: g == 1, b == 0); fall back handled via extra tiles
    g_uniform = all(np.allclose(g, g[0]) for g in gs)
    b_zero = all(not np.any(b) for b in bs)

    # col layout inside one 45-block: [cxr 0:15 | ecg 15:29 pad 29 | ehr 30:43 pads 43:45]
    colmap = {}
    for c in range(CXR):
        colmap[c] = c
    for c in range(ECG):
        colmap[15 + c] = CXR + c
    for c in range(EHR):
        colmap[30 + c] = CXR + ECG + c

    D3 = np.zeros((F * 3 + 1, W * 3))
    for u in range(3):
        for cp, fo in colmap.items():
            D3[u * F:(u + 1) * F, u * W + cp] = D2[fo]
            D3[F * 3, u * W + cp] = c2[fo]

    Kseg = np.array([np.sqrt(E) * (g[0] if g_uniform else 1.0)
                     for (_, E, _), g in zip(segs, gs)])
    kpat = np.tile(Kseg, 9 * B_STATS)                    # [108]
    kseg_rep = np.tile(kpat[None, :], (P, 1))            # [128, 108]

    grepl = brepl = None
    if not g_uniform:
        g48 = np.zeros(W * 3)
        for u in range(3):
            for cp, fo in colmap.items():
                g48[u * W + cp] = np.concatenate(gs)[fo]
        grepl = np.tile(np.tile(g48, 3)[None, :], (P, 1)).astype(np.float32)
    if not b_zero:
        b48 = np.zeros(W * 3)
        for u in range(3):
            for cp, fo in colmap.items():
                b48[u * W + cp] = np.concatenate(bs)[fo]
        brepl = np.tile(np.tile(b48, 3)[None, :], (P, 1)).astype(np.float32)

    return (D3.astype(np.float32), kseg_rep.astype(np.float32), grepl, brepl)


def _slabs(nj, j_slab):
    out = []
    j0 = 0
    while j0 < nj:
        out.append((j0, min(j_slab, nj - j0)))
        j0 += j_slab
    return out


def _build_bass(nj=NJ, j_slab=J_SLAB, general_g=False, general_b=False):
    import concourse.bass as bass
    import concourse.tile as tile
    from concourse import bacc, mybir
    from contextlib import ExitStack

    nc = bacc.Bacc("TRN2", target_bir_lowering=False, debug=False,
                   enable_asserts=False)

    f32 = mybir.dt.float32
    x_dram = nc.dram_tensor("x", [P, nj * F], f32, kind="ExternalInput").ap()
    d3_dram = nc.dram_tensor("d3", [3 * F + 1, 3 * W], f32, kind="ExternalInput").ap()
    id_dram = nc.dram_tensor("ident", [P, P], f32, kind="ExternalInput").ap()
    ones_dram = nc.dram_tensor("ones", [1, 2 * 384], f32, kind="ExternalInput").ap()
    k_dram = nc.dram_tensor("kseg", [P, 27 * B_STATS], f32, kind="ExternalInput").ap()
    g_dram = nc.dram_tensor("grepl", [P, 3 * W * 3], f32, kind="ExternalInput").ap() if general_g else None
    b_dram = nc.dram_tensor("brepl", [P, 3 * W * 3], f32, kind="ExternalInput").ap() if general_b else None
    o_dram = nc.dram_tensor("o", [P, nj * W], f32, kind="ExternalOutput").ap()

    Ln = mybir.ActivationFunctionType.Ln
    Exp = mybir.ActivationFunctionType.Exp
    X = mybir.AxisListType.X

    with tile.TileContext(nc) as tc, ExitStack() as ctx:
        singles = ctx.enter_context(tc.tile_pool(name="singles", bufs=1))
        xs = ctx.enter_context(tc.tile_pool(name="xs", bufs=2))
        os_ = ctx.enter_context(tc.tile_pool(name="os", bufs=2))
        sqp = ctx.enter_context(tc.tile_pool(name="sqp", bufs=3))
        stp = ctx.enter_context(tc.tile_pool(name="stp", bufs=3))
        sgp = ctx.enter_context(tc.tile_pool(name="sgp", bufs=2))
        ivp = ctx.enter_context(tc.tile_pool(name="ivp", bufs=3))
        pxt = ctx.enter_context(tc.tile_pool(name="pxt", bufs=2,
                                             space=bass.MemorySpace.PSUM))
        pos = ctx.enter_context(tc.tile_pool(name="pos", bufs=6,
                                             space=bass.MemorySpace.PSUM))

        ident = singles.tile([P, P], f32)
        d3t = singles.tile([3 * F + 1, 3 * W], f32)
        kst = singles.tile([P, 27 * B_STATS], f32)
        xts = singles.tile([P, 2 * 384], f32)     # two halves, row 126 = ones
        nc.sync.dma_start(ident[:], id_dram)
        nc.sync.dma_start(d3t[:], d3_dram)
        nc.sync.dma_start(kst[:], k_dram)
        if general_g:
            gt = singles.tile([P, 3 * W * 3], f32)
            nc.sync.dma_start(gt[:], g_dram)
        if general_b:
            bt = singles.tile([P, 3 * W * 3], f32)
            nc.sync.dma_start(bt[:], b_dram)
        eps_t = singles.tile([P, 1], f32)
        nc.gpsimd.memset(eps_t[:], LN_EPS)
        nc.sync.dma_start(xts[126:127, :], ones_dram)

        su_ctr = 0
        for (j0, jr) in _slabs(nj, j_slab):
            x = xs.tile([P, j_slab * F], f32)
            nc.sync.dma_start(x[:, :jr * F], x_dram[:, j0 * F:(j0 + jr) * F])
            o = os_.tile([P, j_slab * W], f32)

            n_units = (jr + 2) // 3
            if n_units * 3 > jr:
                nc.vector.memset(x[:, jr * F:n_units * 3 * F], 0.0)
            # superunits: groups of <=3 units sharing one PSUM bank
            sus = []
            t = 0
            while t < n_units:
                sus.append((t, min(3, n_units - t)))
                t += 3
            # stats groups of <= B_STATS superunits
            for g0 in range(0, len(sus), B_STATS):
                grp = sus[g0:g0 + B_STATS]
                ssb = stp.tile([P, 27 * B_STATS], f32)
                osus = []
                wfill = 0
                for k, (t0, nu) in enumerate(grp):
                    xtp = pxt.tile([126, 384], f32)
                    osu = pos.tile([P, 405], f32)
                    for i in range(nu):
                        cj = (t0 + i) * 3 * F
                        nc.tensor.transpose(xtp[0:126, 128 * i:128 * (i + 1)],
                                            x[:, cj:cj + 3 * F], ident[:])
                    h = (su_ctr % 2) * 384
                    nc.scalar.copy(xts[0:126, h:h + 128 * nu],
                                   xtp[0:126, 0:128 * nu])
                    for i in range(nu):
                        nc.tensor.matmul(osu[:, 135 * i:135 * (i + 1)],
                                         xts[0:127, h + 128 * i:h + 128 * (i + 1)],
                                         d3t[:], start=True, stop=True)
                    sq = sqp.tile([P, 405], f32)
                    nc.scalar.square(sq[:, :135 * nu], osu[:, :135 * nu])
                    nc.vector.reduce_sum(
                        ssb[:, 27 * k:27 * k + 9 * nu],
                        sq[:, :135 * nu].rearrange("p (s w) -> p s w", w=15),
                        axis=X)
                    osus.append((osu, t0, nu, 27 * k))
                    wfill = 27 * k + 9 * nu
                    su_ctr += 1
                sig = sgp.tile([P, 27 * B_STATS], f32)
                inv2 = ivp.tile([P, 27 * B_STATS], f32)
                # 1/sqrt(v+eps) = exp(-0.5*ln(v+eps)); ln/exp/square/copy all
                # live in the natural_log_exp_and_others ACT table set
                nc.scalar.activation(sig[:, :wfill], ssb[:, :wfill], Ln,
                                     bias=eps_t[:, 0:1])
                nc.scalar.activation(inv2[:, :wfill], sig[:, :wfill], Exp,
                                     scale=-0.5)
                nc.vector.tensor_mul(inv2[:, :wfill], inv2[:, :wfill],
                                     kst[:, :wfill])
                for (osu, t0, nu, kc) in osus:
                    od = o[:, 405 * (t0 // 3):405 * (t0 // 3) + 135 * nu]
                    bc = inv2[:, kc:kc + 9 * nu].unsqueeze(2).broadcast_to(
                        (P, 9 * nu, 15))
                    nc.vector.tensor_mul(
                        od.rearrange("p (s w) -> p s w", w=15),
                        osu[:, :135 * nu].rearrange("p (s w) -> p s w", w=15),
                        bc)
                    if general_g:
                        nc.vector.tensor_mul(od, od, gt[:, :135 * nu])
                    if general_b:
                        nc.vector.tensor_add(od, od, bt[:, :135 * nu])
            nc.sync.dma_start(o_dram[:, j0 * W:(j0 + jr) * W], o[:, :jr * W])
    nc.compile()
    return nc


def _get_program(general_g, general_b, nj=NJ, j_slab=J_SLAB):
    key = (general_g, general_b, nj, j_slab)
    if key not in _CACHE:
        _CACHE[key] = _build_bass(nj, j_slab, general_g, general_b)
    return _CACHE[key]


def kernel(**inputs):
    from concourse import bass_utils

    d3, kseg, grepl, brepl = _host_consts(inputs)
    xcat = np.concatenate([np.asarray(inputs["cxr_probs"], dtype=np.float32),
                           np.asarray(inputs["ecg_probs"], dtype=np.float32),
                           np.asarray(inputs["ehr_probs"], dtype=np.float32)],
                          axis=1)
    ident = np.eye(P, dtype=np.float32)

    nc = _get_program(grepl is not None, brepl is not None)

    in_maps = []
    for c in range(NCORES):
        m = {"x": np.ascontiguousarray(
                 xcat[c * BC:(c + 1) * BC]).reshape(P, NJ * F),
             "d3": d3, "ident": ident, "kseg": kseg,
             "ones": np.ones((1, 2 * 384), dtype=np.float32)}
        if grepl is not None:
            m["grepl"] = grepl
        if brepl is not None:
            m["brepl"] = brepl
        in_maps.append(m)

    res = bass_utils.run_bass_kernel_spmd(nc, in_maps,
                                          core_ids=list(range(NCORES)))
    O = np.concatenate([r["o"].reshape(BC, W) for r in res.results], axis=0)
    return O[:, 0:CXR], O[:, 15:15 + ECG], O[:, 30:30 + EHR]


# revision 15
# speedup vs baseline: 1.4063x; 1.4063x over previous
"""Trainium2 Bass kernel for nn_CrossModalConceptAttention.

Math: with q_len = kv_len = 1 the attention softmax over a single key is
identically 1, so each branch is an affine map of ctx = concat(cxr, ecg, ehr):

    y_e   = x_e + out_w_e @ (Wv_e @ (kv_w_e @ ctx + kv_b_e) + bv_e) + out_b_e
    out_e = LN(y_e) * g_e + b_e

Everything up to the variance normalization is linear, including the LN mean
subtraction (P = I - 11^T/E).  We fold the whole thing host-side into one
42 -> 42 matrix per branch (scaled by 1/sqrt(E) so the segment sum-of-squares
equals the variance directly), stack the three branches, and run on device:

    per row r:  d = D2 @ ctx_r + c2          (one small matmul)
                var_s = sum(d[seg_s]^2)      (segmented reduce)
                out   = d * Kseg / sqrt(var_s + eps)

Device mapping (per core, rows sharded 8 ways, row p*NJ+j on partition p):
  MM1  (PE):  transpose 3 row-groups of 128 rows at once:
              lhsT = X[128, 126] (42 feats x 3 j), rhs = I[128,128]
              -> X^T stacked [126, 128] in PSUM
  copy (ACT): PSUM -> SBUF, below a persistent row of ones (homogeneous coord)
  MM2  (PE):  lhsT = X^T[127, 128], rhs = D3[127, 144] (block-diag w/ bias row)
              -> d row-major [128 rows, 3 j x 48] in PSUM (48 = 16-padded segs)
  sq   (ACT): square
  red  (DVE): segmented reduce [128, 27, 16] -> [128, 27]
  sqrt (ACT): sigma = sqrt(ssq + eps)   (batched over 4 superunits)
  rcp  (DVE): 1/sigma
  K    (DVE): inv2 = (1/sigma) * Kseg
  mul  (DVE): out = d * broadcast(inv2)  -> SBUF, DMA out (host strips pads)
"""

import numpy as np

B = 2097152
NCORES = 8
BC = B // NCORES            # 262144 rows per core
P = 128                     # SBUF partitions
NJ = BC // P                # 2048 row-groups ("j") per partition
CXR, ECG, EHR = 15, 14, 13
F = CXR + ECG + EHR         # 42 input features
W = 45                      # padded output row width (15 per modality segment)
LN_EPS = 1e-5
J_SLAB = 216                # j-groups per DMA slab
B_STATS = 4                 # superunits per stats batch

_CACHE = {}


def _fold_branch(E, in_w, in_b, out_w, out_b, kv_w, kv_b, lo):
    """Return (D2 rows [E,42], c2 [E]) for one branch, fp64."""
    Wv = in_w[2 * E:].astype(np.float64)
    bv = in_b[2 * E:].astype(np.float64)
    ow = out_w.astype(np.float64)
    ob = out_b.astype(np.float64)
    kw = kv_w.astype(np.float64)
    kb = kv_b.astype(np.float64)
    A = ow @ Wv @ kw                       # [E, 42]
    c = ow @ (Wv @ kb + bv) + ob           # [E]
    M = A.copy()
    M[:, lo:lo + E] += np.eye(E)           # residual x_e
    Pc = np.eye(E) - 1.0 / E               # LN centering
    s = 1.0 / np.sqrt(E)                   # so sum(d^2) == var
    return (Pc @ M) * s, (Pc @ c) * s


def _host_consts(inputs):
    segs = [("cxr", CXR, 0), ("ecg", ECG, CXR), ("ehr", EHR, CXR + ECG)]
    D2 = np.zeros((F, F))
    c2 = np.zeros(F)
    gs, bs = [], []
    for name, E, lo in segs:
        D2[lo:lo + E], c2[lo:lo + E] = _fold_branch(
            E,
            np.asarray(inputs[f"in_w_{name}"]), np.asarray(inputs[f"in_b_{name}"]),
            np.asarray(inputs[f"out_w_{name}"]), np.asarray(inputs[f"out_b_{name}"]),
            np.asarray(inputs[f"kv_w_{name}"]), np.asarray(inputs[f"kv_b_{name}"]),
            lo)
        gs.append(np.asarray(inputs[f"ln_g_{name}"], dtype=np.float64))
        bs.append(np.asarray(inputs[f"ln_b_{name}"], dtype=np.float64))

    # fast path requires per-segment-uniform ln_g and zero ln_b (true for
    # this model's setup: g == 1, b == 0); fall back handled via extra tiles
    g_uniform = all(np.allclose(g, g[0]) for g in gs)
    b_zero = all(not np.any(b) for b in bs)

    # col layout inside one 45-block: [cxr 0:15 | ecg 15:29 pad 29 | ehr 30:43 pads 43:45]
    colmap = {}
    for c in range(CXR):
        colmap[c] = c
    for c in range(ECG):
        colmap[15 + c] = CXR + c
    for c in range(EHR):
        colmap[30 + c] = CXR + ECG + c

    D3 = np.zeros((F * 3 + 1, W * 3))
    for u in range(3):
        for cp, fo in colmap.items():
            D3[u * F:(u + 1) * F, u * W + cp] = D2[fo]
            D3[F * 3, u * W + cp] = c2[fo]

    Kseg = np.array([np.sqrt(E) * (g[0] if g_uniform else 1.0)
                     for (_, E, _), g in zip(segs, gs)])
    kpat = np.tile(Kseg, 9 * B_STATS)                    # [108]
    kseg_rep = np.tile(kpat[None, :], (P, 1))            # [128, 108]

    grepl = brepl = None
    if not g_uniform:
        g48 = np.zeros(W * 3)
        for u in range(3):
            for cp, fo in colmap.items():
                g48[u * W + cp] = np.concatenate(gs)[fo]
        grepl = np.tile(np.tile(g48, 3)[None, :], (P, 1)).astype(np.float32)
    if not b_zero:
        b48 = np.zeros(W * 3)
        for u in range(3):
            for cp, fo in colmap.items():
                b48[u * W + cp] = np.concatenate(bs)[fo]
        brepl = np.tile(np.tile(b48, 3)[None, :], (P, 1)).astype(np.float32)

    return (D3.astype(np.float32), kseg_rep.astype(np.float32), grepl, brepl)


def _slabs(nj, j_slab):
    out = []
    j0 = 0
    while j0 < nj:
        out.append((j0, min(j_slab, nj - j0)))
        j0 += j_slab
    return out


def _build_bass(nj=NJ, j_slab=J_SLAB, general_g=False, general_b=False):
    import concourse.bass as bass
    import concourse.tile as tile
    from concourse import bacc, mybir
    from contextlib import ExitStack

    nc = bacc.Bacc("TRN2", target_bir_lowering=False, debug=False,
                   enable_asserts=False)

    f32 = mybir.dt.float32
    x_dram = nc.dram_tensor("x", [P, nj * F], f32, kind="ExternalInput").ap()
    d3_dram = nc.dram_tensor("d3", [3 * F + 1, 3 * W], f32, kind="ExternalInput").ap()
    id_dram = nc.dram_tensor("ident", [P, P], f32, kind="ExternalInput").ap()
    ones_dram = nc.dram_tensor("ones", [1, 2 * 384], f32, kind="ExternalInput").ap()
    k_dram = nc.dram_tensor("kseg", [P, 27 * B_STATS], f32, kind="ExternalInput").ap()
    g_dram = nc.dram_tensor("grepl", [P, 3 * W * 3], f32, kind="ExternalInput").ap() if general_g else None
    b_dram = nc.dram_tensor("brepl", [P, 3 * W * 3], f32, kind="ExternalInput").ap() if general_b else None
    o_dram = nc.dram_tensor("o", [P, nj * W], f32, kind="ExternalOutput").ap()

    Sqrt = mybir.ActivationFunctionType.Sqrt
    X = mybir.AxisListType.X

    with tile.TileContext(nc) as tc, ExitStack() as ctx:
        singles = ctx.enter_context(tc.tile_pool(name="singles", bufs=1))
        xs = ctx.enter_context(tc.tile_pool(name="xs", bufs=2))
        os_ = ctx.enter_context(tc.tile_pool(name="os", bufs=2))
        sqp = ctx.enter_context(tc.tile_pool(name="sqp", bufs=3))
        stp = ctx.enter_context(tc.tile_pool(name="stp", bufs=3))
        sgp = ctx.enter_context(tc.tile_pool(name="sgp", bufs=2))
        ivp = ctx.enter_context(tc.tile_pool(name="ivp", bufs=3))
        pxt = ctx.enter_context(tc.tile_pool(name="pxt", bufs=2,
                                             space=bass.MemorySpace.PSUM))
        pos = ctx.enter_context(tc.tile_pool(name="pos", bufs=6,
                                             space=bass.MemorySpace.PSUM))

        ident = singles.tile([P, P], f32)
        d3t = singles.tile([3 * F + 1, 3 * W], f32)
        kst = singles.tile([P, 27 * B_STATS], f32)
        xts = singles.tile([P, 2 * 384], f32)     # two halves, row 126 = ones
        nc.sync.dma_start(ident[:], id_dram)
        nc.sync.dma_start(d3t[:], d3_dram)
        nc.sync.dma_start(kst[:], k_dram)
        if general_g:
            gt = singles.tile([P, 3 * W * 3], f32)
            nc.sync.dma_start(gt[:], g_dram)
        if general_b:
            bt = singles.tile([P, 3 * W * 3], f32)
            nc.sync.dma_start(bt[:], b_dram)
        eps_t = singles.tile([P, 1], f32)
        nc.gpsimd.memset(eps_t[:], LN_EPS)
        nc.sync.dma_start(xts[126:127, :], ones_dram)

        su_ctr = 0
        for (j0, jr) in _slabs(nj, j_slab):
            x = xs.tile([P, j_slab * F], f32)
            nc.sync.dma_start(x[:, :jr * F], x_dram[:, j0 * F:(j0 + jr) * F])
            o = os_.tile([P, j_slab * W], f32)

            n_units = (jr + 2) // 3
            if n_units * 3 > jr:
                nc.vector.memset(x[:, jr * F:n_units * 3 * F], 0.0)
            # superunits: groups of <=3 units sharing one PSUM bank
            sus = []
            t = 0
            while t < n_units:
                sus.append((t, min(3, n_units - t)))
                t += 3
            # stats groups of <= B_STATS superunits
            for g0 in range(0, len(sus), B_STATS):
                grp = sus[g0:g0 + B_STATS]
                ssb = stp.tile([P, 27 * B_STATS], f32)
                osus = []
                wfill = 0
                for k, (t0, nu) in enumerate(grp):
                    xtp = pxt.tile([126, 384], f32)
                    osu = pos.tile([P, 405], f32)
                    for i in range(nu):
                        cj = (t0 + i) * 3 * F
                        nc.tensor.transpose(xtp[0:126, 128 * i:128 * (i + 1)],
                                            x[:, cj:cj + 3 * F], ident[:])
                    h = (su_ctr % 2) * 384
                    nc.scalar.copy(xts[0:126, h:h + 128 * nu],
                                   xtp[0:126, 0:128 * nu])
                    for i in range(nu):
                        nc.tensor.matmul(osu[:, 135 * i:135 * (i + 1)],
                                         xts[0:127, h + 128 * i:h + 128 * (i + 1)],
                                         d3t[:], start=True, stop=True)
                    sq = sqp.tile([P, 405], f32)
                    nc.scalar.square(sq[:, :135 * nu], osu[:, :135 * nu])
                    nc.vector.reduce_sum(
                        ssb[:, 27 * k:27 * k + 9 * nu],
                        sq[:, :135 * nu].rearrange("p (s w) -> p s w", w=15),
                        axis=X)
                    osus.append((osu, t0, nu, 27 * k))
                    wfill = 27 * k + 9 * nu
                    su_ctr += 1
                sig = sgp.tile([P, 27 * B_STATS], f32)
                inv2 = ivp.tile([P, 27 * B_STATS], f32)
                # sigma = sqrt(v + eps) on ACT (sqrt/square/copy share one
                # table set); inv2 = Kseg / sigma fused on the idle GPSIMD
                nc.scalar.activation(sig[:, :wfill], ssb[:, :wfill], Sqrt,
                                     bias=eps_t[:, 0:1])
                nc.vector.reciprocal(inv2[:, :wfill], sig[:, :wfill])
                nc.gpsimd.tensor_mul(inv2[:, :wfill], inv2[:, :wfill],
                                     kst[:, :wfill])
                for (osu, t0, nu, kc) in osus:
                    od = o[:, 405 * (t0 // 3):405 * (t0 // 3) + 135 * nu]
                    bc = inv2[:, kc:kc + 9 * nu].unsqueeze(2).broadcast_to(
                        (P, 9 * nu, 15))
                    nc.vector.tensor_mul(
                        od.rearrange("p (s w) -> p s w", w=15),
                        osu[:, :135 * nu].rearrange("p (s w) -> p s w", w=15),
                        bc)
                    if general_g:
                        nc.vector.tensor_mul(od, od, gt[:, :135 * nu])
                    if general_b:
                        nc.vector.tensor_add(od, od, bt[:, :135 * nu])
            nc.sync.dma_start(o_dram[:, j0 * W:(j0 + jr) * W], o[:, :jr * W])
    nc.compile()
    return nc


def _get_program(general_g, general_b, nj=NJ, j_slab=J_SLAB):
    key = (general_g, general_b, nj, j_slab)
    if key not in _CACHE:
        _CACHE[key] = _build_bass(nj, j_slab, general_g, general_b)
    return _CACHE[key]


def kernel(**inputs):
    from concourse import bass_utils

    d3, kseg, grepl, brepl = _host_consts(inputs)
    xcat = np.concatenate([np.asarray(inputs["cxr_probs"], dtype=np.float32),
                           np.asarray(inputs["ecg_probs"], dtype=np.float32),
                           np.asarray(inputs["ehr_probs"], dtype=np.float32)],
                          axis=1)
    ident = np.eye(P, dtype=np.float32)

    nc = _get_program(grepl is not None, brepl is not None)

    in_maps = []
    for c in range(NCORES):
        m = {"x": np.ascontiguousarray(
                 xcat[c * BC:(c + 1) * BC]).reshape(P, NJ * F),
             "d3": d3, "ident": ident, "kseg": kseg,
             "ones": np.ones((1, 2 * 384), dtype=np.float32)}
        if grepl is not None:
            m["grepl"] = grepl
        if brepl is not None:
            m["brepl"] = brepl
        in_maps.append(m)

    res = bass_utils.run_bass_kernel_spmd(nc, in_maps,
                                          core_ids=list(range(NCORES)))
    O = np.concatenate([r["o"].reshape(BC, W) for r in res.results], axis=0)
    return O[:, 0:CXR], O[:, 15:15 + ECG], O[:, 30:30 + EHR]
